# revision 10
# baseline (speedup 1.0000x reference)
"""Trainium2 Bass kernel for the affine-transformer backsubstitution chain.

reference semantics (D=2048, L=8):
    Al = Au = A; bl = bu = b
    for s in 0..L-1 (history reversed):
        Al' = relu(Al) @ dAl + min(Al,0) @ dAu
        bl' = relu(Al) @ dbl + min(Al,0) @ dbu + bl
        Au' = relu(Au) @ dAu + min(Au,0) @ dAl
        bu' = relu(Au) @ dbu + min(Au,0) @ dbl + bu
    lower = relu(Al) @ lower_in + min(Al,0) @ upper_in + bl
    upper = relu(Au) @ upper_in + min(Au,0) @ lower_in + bu

Sharding: rows of Al/Au across 8 cores (256 rows each), history replicated.
Per core the state is kept TRANSPOSED ([2048 k-partitions, 256 m-free]) so the
history matrices act directly as matmul weights (out = lhsT.T @ rhs), and the
clamped copies are the state:
    mvA[k] = [ relu(AlT)[k] | min(AuT,0)[k] ]   (pairs with dAl weight tiles)
    mvB[k] = [ min(AlT,0)[k] | relu(AuT)[k] ]   (pairs with dAu weight tiles)
One [128,512] PSUM per output chunk then accumulates both chains at once:
    psum[:, :256] = sum_k dAl[k,n]·relu(AlT) + dAu[k,n]·min(AlT,0) = new AlT
    psum[:, 256:] = sum_k dAl[k,n]·min(AuT,0) + dAu[k,n]·relu(AuT) = new AuT
Compute dtype bf16 (fp32 PSUM accumulation); rel err vs fp32 ≈ 2.5e-3.

The bias chain and the final concretization are m=1 matvecs against the same
state tiles (mvA pairs with dbl/lower_in, mvB with dbu/upper_in). They run as
128x32 column-tiled matmuls — tile_position=(0,32g), g = chunk%4 — so four
stream concurrently in separate column groups of the PE array, and ALL of them
(8 steps x 32 + final 32) accumulate into one PSUM bank on partition rows
{0,32,64,96}; a single DVE pass at the end sums the four rows and adds b.
This costs ~8 serialized matmul slots per step instead of 16 (fp8 DoubleRow)
or 32 (naive), and needs no fp8 shadow state.

PE work per core: 4096 main matmuls x ~220 ns (N=512 stream at 2.4 GHz + NX
dispatch) + ~160 col-tiled matvecs in ~40 4-way groups ≈ 910 µs warm;
runs land ~1.13 ms when the chip drops to its 2.0 GHz P0 power state.
"""

import numpy as np
import ml_dtypes

L = 8
D = 2048
NCORES = 8
RPC = D // NCORES  # 256 rows per core
P = 128
KC = D // P  # 16 partition chunks
W = 2 * RPC  # 512: concatenated moving width

BF16 = ml_dtypes.bfloat16

_nc_cache = {}


def _build():
    from concourse import bacc
    import concourse.tile as tile
    import concourse.mybir as mybir

    dt = mybir.dt
    nc = bacc.Bacc()

    at0 = nc.dram_tensor("at0", [KC, P, RPC], dt.bfloat16, kind="ExternalInput")
    hist = nc.dram_tensor("hist", [L, KC // 2, 2, 2, P, D], dt.bfloat16, kind="ExternalInput")
    # hbv[p, (s*2+f)*KC + i] = (dbl if f==0 else dbu)[s, i*128+p]: per-chunk
    # bias-vector columns used as m=1 stationary weights.
    hbv = nc.dram_tensor("hbv", [P, L * 2 * KC], dt.bfloat16, kind="ExternalInput")
    fin = nc.dram_tensor("fin", [P, 2 * KC], dt.bfloat16, kind="ExternalInput")
    b2 = nc.dram_tensor("b2", [1, W], dt.float32, kind="ExternalInput")
    out = nc.dram_tensor("out", [1, W], dt.float32, kind="ExternalOutput")

    with tile.TileContext(nc) as tc:
        with (
            tc.tile_pool(name="state", bufs=1) as spool,
            tc.tile_pool(name="wts", bufs=4) as wpool,
            tc.tile_pool(name="consts", bufs=1) as cpool,
            tc.tile_pool(name="bias", bufs=1) as bpool,
            tc.tile_pool(name="psum", bufs=7, space="PSUM") as ppool,
            tc.tile_pool(name="psumb", bufs=1, space="PSUM") as pbpool,
        ):
            mvA = [spool.tile([P, KC * W], dt.bfloat16, tag=f"mvA{i}", name=f"mvA{i}") for i in range(2)]
            mvB = [spool.tile([P, KC * W], dt.bfloat16, tag=f"mvB{i}", name=f"mvB{i}") for i in range(2)]
            hbvt = cpool.tile([P, L * 2 * KC], dt.bfloat16, tag="hbvt")
            fint = cpool.tile([P, 2 * KC], dt.bfloat16, tag="fint")
            b2t = bpool.tile([1, W], dt.float32, tag="b2t")

            # One PSUM bank accumulates every m=1 matvec of the kernel (bias
            # chain + final concretization) on partition rows {0,32,64,96}.
            pbias = pbpool.tile([P, W], dt.float32, tag="pb", name="pb")

            # PE warmup: a few cheap matmuls on a zeroed tile bridge the
            # initial DMA window without delaying the first real matmul.
            warm = cpool.tile([P, W], dt.bfloat16, tag="warm")
            nc.vector.memset(warm[:], 0.0)
            pw = ppool.tile([P, W], dt.float32, tag="ps", name="pw")
            for i in range(16):
                nc.tensor.matmul(pw[:, :P], warm[:, :P], warm[:, :P], start=True, stop=True)

            # Startup loads in consumption order, spread over the sync and
            # scalar queues (state chunk-pairs interleaved with the first
            # stripe's k-quarters; each dma_start costs ~0.7 µs of sequencer
            # dispatch). GpSimd stays free for its half of the clamps; its
            # const loads are emitted after those.
            stg = cpool.tile([P, KC, RPC], dt.bfloat16, tag="stg", name="stg")
            stripes = {}
            st00 = wpool.tile([P, 2, 2, D], dt.bfloat16, tag="stripe", name="stripe")
            stripes[(0, 0)] = st00
            h00 = hist[0, 0]
            st01 = wpool.tile([P, 2, 2, D], dt.bfloat16, tag="stripe", name="stripe")
            stripes[(0, 1)] = st01

            def load_pair(q, eng):
                eng.dma_start(
                    stg[:, 2 * q : 2 * (q + 1), :],
                    at0[2 * q : 2 * (q + 1)].rearrange("k p r -> p k r"),
                )

            for g in range(4):
                load_pair(2 * g, nc.sync)
                load_pair(2 * g + 1, nc.scalar)
                sl = slice(g * D // 4, (g + 1) * D // 4)
                nc.sync.dma_start(
                    st00[:, :, :, sl], h00[:, :, :, sl].rearrange("jh t p f -> p jh t f")
                )
                if g == 0:
                    nc.scalar.dma_start(
                        st01[:], hist[0, 1].rearrange("jh t p f -> p jh t f")
                    )

            # Step-0 state: Al = Au = A, so only mvA = [relu(AT) | min(AT,0)]
            # is materialized (the B-family reads its halves swapped); DVE and
            # GpSimd each clamp half the chunks.
            for i in range(KC):
                o = i * W
                s_i = stg[:, i, :]
                eng = nc.vector if i % 2 == 0 else nc.gpsimd
                eng.tensor_scalar_max(mvA[0][:, o : o + RPC], s_i, 0.0)
                eng.tensor_scalar_min(mvA[0][:, o + RPC : o + W], s_i, 0.0)
            nc.gpsimd.dma_start(hbvt[:], hbv[:])
            nc.gpsimd.dma_start(fint[:], fin[:])
            nc.gpsimd.dma_start(b2t[:], b2[:])

            for s in range(L):
                cur, nxt = s % 2, (s + 1) % 2
                A, B = mvA[cur], mvB[cur]
                An, Bn = mvA[nxt], mvB[nxt]
                for jp in range(KC // 2):
                    if (s, jp) in stripes:
                        stripe = stripes.pop((s, jp))
                    else:
                        stripe = wpool.tile([P, 2, 2, D], dt.bfloat16, tag="stripe", name="stripe")
                        nc.sync.dma_start(
                            stripe[:], hist[s, jp].rearrange("jh t p f -> p jh t f")
                        )
                    for jh in range(2):
                        j = 2 * jp + jh
                        ps = ppool.tile([P, W], dt.float32, tag="ps", name="ps")
                        for i in range(KC):
                            wA = stripe[:, jh, 0, i * P : (i + 1) * P]
                            wB = stripe[:, jh, 1, i * P : (i + 1) * P]
                            if s == 0:
                                # mvB isn't materialized at step 0 (Al = Au):
                                # the B-family reads mvA's halves swapped via
                                # two n=256 matmuls. The i==KC-1 A-matmul is
                                # reordered last to carry the full-width stop.
                                relu_h = A[:, i * W : i * W + RPC]
                                min_h = A[:, i * W + RPC : (i + 1) * W]
                                mms = [
                                    (ps[:], wA, A[:, i * W : (i + 1) * W], i == 0, i == KC - 1),
                                    (ps[:, :RPC], wB, min_h, False, False),
                                    (ps[:, RPC:], wB, relu_h, False, False),
                                ]
                                if i == KC - 1:
                                    mms = mms[1:] + mms[:1]
                                for o_ap, w_ap, r_ap, st, sp in mms:
                                    nc.tensor.matmul(o_ap, w_ap, r_ap, start=st, stop=sp)
                            else:
                                nc.tensor.matmul(
                                    ps[:],
                                    wA,
                                    A[:, i * W : (i + 1) * W],
                                    start=(i == 0),
                                    stop=False,
                                )
                                nc.tensor.matmul(
                                    ps[:],
                                    wB,
                                    B[:, i * W : (i + 1) * W],
                                    start=False,
                                    stop=(i == KC - 1),
                                )
                        h = RPC
                        o = j * W
                        nc.vector.tensor_scalar_max(An[:, o : o + h], ps[:, :h], 0.0)
                        nc.vector.tensor_scalar_min(Bn[:, o : o + h], ps[:, :h], 0.0)
                        nc.vector.tensor_scalar_max(Bn[:, o + h : o + W], ps[:, h:], 0.0)
                        nc.vector.tensor_scalar_min(An[:, o + h : o + W], ps[:, h:], 0.0)
                # bias chain: column-tiled m=1 matvecs, four concurrent in
                # separate 32-column PE groups, accumulating into pbias rows
                # {0,32,64,96}. A-family (rhs mvA) pairs with dbl, B-family
                # (rhs mvB) with dbu.
                for f, rhs_t in enumerate((A, B)):
                    base = (s * 2 + f) * KC
                    for i in range(KC):
                        g = 32 * (i % 4)
                        vcol = hbvt[:, base + i : base + i + 1]
                        if s == 0 and f == 1:
                            nc.tensor.matmul(
                                pbias[g : g + 1, :RPC],
                                vcol,
                                A[:, i * W + RPC : (i + 1) * W],
                                start=False, stop=False, tile_position=(0, g),
                            )
                            nc.tensor.matmul(
                                pbias[g : g + 1, RPC:],
                                vcol,
                                A[:, i * W : i * W + RPC],
                                start=False, stop=False, tile_position=(0, g),
                            )
                        else:
                            nc.tensor.matmul(
                                pbias[g : g + 1, :],
                                vcol,
                                rhs_t[:, i * W : (i + 1) * W],
                                start=(s == 0 and f == 0 and i < 4),
                                stop=False,
                                tile_position=(0, g),
                            )

            # final concretization against the input box, same col-tiled
            # accumulation: mvA pairs with lower_in, mvB with upper_in.
            Af, Bf = mvA[L % 2], mvB[L % 2]
            for f, rhs_t in enumerate((Af, Bf)):
                for i in range(KC):
                    g = 32 * (i % 4)
                    nc.tensor.matmul(
                        pbias[g : g + 1, :],
                        fint[:, f * KC + i : f * KC + i + 1],
                        rhs_t[:, i * W : (i + 1) * W],
                        start=False,
                        stop=(f == 1 and i >= KC - 4),
                        tile_position=(0, g),
                    )

            # res = sum of the four accumulator rows + b (one PSUM operand
            # per DVE instruction)
            acc = bpool.tile([1, W], dt.float32, tag="acc")
            res = bpool.tile([1, W], dt.float32, tag="res")
            nc.vector.tensor_add(acc[:], b2t[:], pbias[0:1, :])
            nc.vector.tensor_add(acc[:], acc[:], pbias[32:33, :])
            nc.vector.tensor_add(acc[:], acc[:], pbias[64:65, :])
            nc.vector.tensor_add(res[:], acc[:], pbias[96:97, :])
            nc.sync.dma_start(out[:], res[:])

    nc.finalize()
    return nc


def _get_nc():
    if "nc" not in _nc_cache:
        _nc_cache["nc"] = _build()
    return _nc_cache["nc"]


def _prep_inputs(A, b, hist_Al, hist_Au, hist_bl, hist_bu, lower_in, upper_in):
    A = np.asarray(A, dtype=np.float32)
    b = np.asarray(b, dtype=np.float32)
    hal = np.asarray(hist_Al, dtype=np.float32)[::-1]
    hau = np.asarray(hist_Au, dtype=np.float32)[::-1]
    hbl = np.asarray(hist_bl, dtype=np.float32)[::-1]
    hbu = np.asarray(hist_bu, dtype=np.float32)[::-1]
    lower_in = np.asarray(lower_in, dtype=np.float32)
    upper_in = np.asarray(upper_in, dtype=np.float32)

    # hist[s, j, t, p, i*P + n] = h_t[s, i*P + p, j*P + n], paired over j
    hist = np.empty([L, KC, 2, P, D], dtype=BF16)
    for t, h in enumerate((hal, hau)):
        hist[:, :, t] = (
            h.reshape(L, KC, P, KC, P).transpose(0, 3, 2, 1, 4).reshape(L, KC, P, D)
        )
    hist = hist.reshape(L, KC // 2, 2, 2, P, D)

    # hbv[p, (s*2+f)*KC + i] = (dbl, dbu)[f][s, i*P + p]
    hbv = (
        np.stack([hbl, hbu], axis=1)  # [L, 2, D]
        .reshape(L * 2 * KC, P)
        .T.astype(BF16)
    )
    hbv = np.ascontiguousarray(hbv)

    # fin[p, t*KC + i]: t=0 lower_in, t=1 upper_in
    fin = (
        np.stack([lower_in.reshape(KC, P), upper_in.reshape(KC, P)], axis=0)
        .transpose(2, 0, 1)
        .reshape(P, 2 * KC)
        .astype(BF16)
    )

    in_maps = []
    for c in range(NCORES):
        At = np.ascontiguousarray(A[c * RPC : (c + 1) * RPC].T)  # [D, RPC]
        at0 = At.reshape(KC, P, RPC).astype(BF16)
        b_blk = b[c * RPC : (c + 1) * RPC]
        b2 = np.concatenate([b_blk, b_blk]).reshape(1, W).astype(np.float32)
        in_maps.append(
            {
                "at0": at0,
                "hist": hist,
                "hbv": hbv,
                "fin": fin,
                "b2": b2,
            }
        )
    return in_maps


def _run(in_maps, trace=False):
    from concourse.bass_utils import run_bass_kernel_spmd

    nc = _get_nc()
    return run_bass_kernel_spmd(
        nc, in_maps, core_ids=list(range(NCORES)), trace=trace
    )


def kernel(A, b, hist_Al, hist_Au, hist_bl, hist_bu, lower_in, upper_in):
    in_maps = _prep_inputs(
        A, b, hist_Al, hist_Au, hist_bl, hist_bu, lower_in, upper_in
    )
    res = _run(in_maps, trace=False)
    lower = np.concatenate([res.results[c]["out"][0, :RPC] for c in range(NCORES)])
    upper = np.concatenate([res.results[c]["out"][0, RPC:] for c in range(NCORES)])
    return lower.astype(np.float32), upper.astype(np.float32)


# revision 11
# speedup vs baseline: 1.0464x; 1.0464x over previous
"""Trainium2 Bass kernel for the affine-transformer backsubstitution chain.

reference semantics (D=2048, L=8):
    Al = Au = A; bl = bu = b
    for s in 0..L-1 (history reversed):
        Al' = relu(Al) @ dAl + min(Al,0) @ dAu
        bl' = relu(Al) @ dbl + min(Al,0) @ dbu + bl
        Au' = relu(Au) @ dAu + min(Au,0) @ dAl
        bu' = relu(Au) @ dbu + min(Au,0) @ dbl + bu
    lower = relu(Al) @ lower_in + min(Al,0) @ upper_in + bl
    upper = relu(Au) @ upper_in + min(Au,0) @ lower_in + bu

Sharding: rows of Al/Au across 8 cores (256 rows each), history replicated.
Per core the state is kept TRANSPOSED ([2048 k-partitions, 256 m-free]) so the
history matrices act directly as matmul weights (out = lhsT.T @ rhs), and the
clamped copies are the state:
    mvA[k] = [ relu(AlT)[k] | min(AuT,0)[k] ]   (pairs with dAl weight tiles)
    mvB[k] = [ min(AlT,0)[k] | relu(AuT)[k] ]   (pairs with dAu weight tiles)
One [128,512] PSUM per output chunk then accumulates both chains at once:
    psum[:, :256] = sum_k dAl[k,n]·relu(AlT) + dAu[k,n]·min(AlT,0) = new AlT
    psum[:, 256:] = sum_k dAl[k,n]·min(AuT,0) + dAu[k,n]·relu(AuT) = new AuT
Compute dtype bf16 (fp32 PSUM accumulation); rel err vs fp32 ≈ 2.5e-3.

The bias chain and the final concretization are m=1 matvecs against the same
state tiles (mvA pairs with dbl/lower_in, mvB with dbu/upper_in). They run as
128x32 column-tiled matmuls — tile_position=(0,32g), g = chunk%4 — so four
stream concurrently in separate column groups of the PE array, and ALL of them
(8 steps x 32 + final 32) accumulate into one PSUM bank on partition rows
{0,32,64,96}; a single DVE pass at the end sums the four rows and adds b.
This costs ~8 serialized matmul slots per step instead of 16 (fp8 DoubleRow)
or 32 (naive), and needs no fp8 shadow state.

PE work per core: 4096 main matmuls x ~220 ns (N=512 stream at 2.4 GHz + NX
dispatch) + ~160 col-tiled matvecs in ~40 4-way groups ≈ 910 µs warm;
runs land ~1.13 ms when the chip drops to its 2.0 GHz P0 power state.
"""

import numpy as np
import ml_dtypes

L = 8
D = 2048
NCORES = 8
RPC = D // NCORES  # 256 rows per core
P = 128
KC = D // P  # 16 partition chunks
W = 2 * RPC  # 512: concatenated moving width

BF16 = ml_dtypes.bfloat16

_nc_cache = {}


def _build():
    from concourse import bacc
    import concourse.tile as tile
    import concourse.mybir as mybir

    dt = mybir.dt
    nc = bacc.Bacc()

    at0 = nc.dram_tensor("at0", [KC, P, RPC], dt.bfloat16, kind="ExternalInput")
    hist = nc.dram_tensor("hist", [L, KC // 2, 2, 2, P, D], dt.bfloat16, kind="ExternalInput")
    # hbv[p, (s*2+f)*KC + i] = (dbl if f==0 else dbu)[s, i*128+p]: per-chunk
    # bias-vector columns used as m=1 stationary weights.
    hbv = nc.dram_tensor("hbv", [P, L * 2 * KC], dt.bfloat16, kind="ExternalInput")
    fin = nc.dram_tensor("fin", [P, 2 * KC], dt.bfloat16, kind="ExternalInput")
    b2 = nc.dram_tensor("b2", [1, W], dt.float32, kind="ExternalInput")
    out = nc.dram_tensor("out", [1, W], dt.float32, kind="ExternalOutput")

    with tile.TileContext(nc) as tc:
        with (
            tc.tile_pool(name="state", bufs=1) as spool,
            tc.tile_pool(name="wts", bufs=4) as wpool,
            tc.tile_pool(name="consts", bufs=1) as cpool,
            tc.tile_pool(name="bias", bufs=1) as bpool,
            tc.tile_pool(name="psum", bufs=7, space="PSUM") as ppool,
            tc.tile_pool(name="psumb", bufs=1, space="PSUM") as pbpool,
        ):
            mvA = [spool.tile([P, KC * W], dt.bfloat16, tag=f"mvA{i}", name=f"mvA{i}") for i in range(2)]
            mvB = [spool.tile([P, KC * W], dt.bfloat16, tag=f"mvB{i}", name=f"mvB{i}") for i in range(2)]
            hbvt = cpool.tile([P, L * 2 * KC], dt.bfloat16, tag="hbvt")
            fint = cpool.tile([P, 2 * KC], dt.bfloat16, tag="fint")
            b2t = bpool.tile([1, W], dt.float32, tag="b2t")

            # One PSUM bank accumulates every m=1 matvec of the kernel (bias
            # chain + final concretization) on partition rows {0,32,64,96}.
            pbias = pbpool.tile([P, W], dt.float32, tag="pb", name="pb")

            # PE warmup: a few cheap matmuls on a zeroed tile bridge the
            # initial DMA window without delaying the first real matmul.
            warm = cpool.tile([P, W], dt.bfloat16, tag="warm")
            nc.vector.memset(warm[:], 0.0)
            pw = ppool.tile([P, W], dt.float32, tag="ps", name="pw")
            for i in range(16):
                nc.tensor.matmul(pw[:, :P], warm[:, :P], warm[:, :P], start=True, stop=True)

            # Startup loads: state chunk-pairs get the sync queue to
            # themselves (each dma_start costs ~0.7 µs of sequencer dispatch,
            # and a pair lands about every 0.7 µs — matching the PE's step-0
            # consumption rate); the first two stripes and the consts go on
            # gpsimd. GpSimd compute is useless here (~4.6 µs per clamp), so
            # the step-0 clamps split relu→ScalarE / min→DVE instead.
            stg = cpool.tile([P, KC, RPC], dt.bfloat16, tag="stg", name="stg")
            stripes = {}
            st00 = wpool.tile([P, 2, 2, D], dt.bfloat16, tag="stripe", name="stripe")
            stripes[(0, 0)] = st00
            h00 = hist[0, 0]
            st01 = wpool.tile([P, 2, 2, D], dt.bfloat16, tag="stripe", name="stripe")
            stripes[(0, 1)] = st01

            for q in range(KC // 2):
                nc.sync.dma_start(
                    stg[:, 2 * q : 2 * (q + 1), :],
                    at0[2 * q : 2 * (q + 1)].rearrange("k p r -> p k r"),
                )
            for g in range(4):
                sl = slice(g * D // 4, (g + 1) * D // 4)
                nc.gpsimd.dma_start(
                    st00[:, :, :, sl], h00[:, :, :, sl].rearrange("jh t p f -> p jh t f")
                )
            nc.gpsimd.dma_start(st01[:], hist[0, 1].rearrange("jh t p f -> p jh t f"))
            nc.gpsimd.dma_start(hbvt[:], hbv[:])
            nc.gpsimd.dma_start(fint[:], fin[:])
            nc.gpsimd.dma_start(b2t[:], b2[:])

            # Step-0 state: Al = Au = A, so only mvA = [relu(AT) | min(AT,0)]
            # is materialized (the B-family reads its halves swapped).
            relu_f = mybir.ActivationFunctionType.Relu
            for i in range(KC):
                o = i * W
                s_i = stg[:, i, :]
                nc.scalar.activation(mvA[0][:, o : o + RPC], s_i, relu_f)
                nc.vector.tensor_scalar_min(mvA[0][:, o + RPC : o + W], s_i, 0.0)

            for s in range(L):
                cur, nxt = s % 2, (s + 1) % 2
                A, B = mvA[cur], mvB[cur]
                An, Bn = mvA[nxt], mvB[nxt]
                for jp in range(KC // 2):
                    if (s, jp) in stripes:
                        stripe = stripes.pop((s, jp))
                    else:
                        stripe = wpool.tile([P, 2, 2, D], dt.bfloat16, tag="stripe", name="stripe")
                        nc.sync.dma_start(
                            stripe[:], hist[s, jp].rearrange("jh t p f -> p jh t f")
                        )
                    for jh in range(2):
                        j = 2 * jp + jh
                        ps = ppool.tile([P, W], dt.float32, tag="ps", name="ps")
                        for i in range(KC):
                            wA = stripe[:, jh, 0, i * P : (i + 1) * P]
                            wB = stripe[:, jh, 1, i * P : (i + 1) * P]
                            if s == 0:
                                # mvB isn't materialized at step 0 (Al = Au):
                                # the B-family reads mvA's halves swapped via
                                # two n=256 matmuls. The i==KC-1 A-matmul is
                                # reordered last to carry the full-width stop.
                                relu_h = A[:, i * W : i * W + RPC]
                                min_h = A[:, i * W + RPC : (i + 1) * W]
                                mms = [
                                    (ps[:], wA, A[:, i * W : (i + 1) * W], i == 0, i == KC - 1),
                                    (ps[:, :RPC], wB, min_h, False, False),
                                    (ps[:, RPC:], wB, relu_h, False, False),
                                ]
                                if i == KC - 1:
                                    mms = mms[1:] + mms[:1]
                                for o_ap, w_ap, r_ap, st, sp in mms:
                                    nc.tensor.matmul(o_ap, w_ap, r_ap, start=st, stop=sp)
                            else:
                                nc.tensor.matmul(
                                    ps[:],
                                    wA,
                                    A[:, i * W : (i + 1) * W],
                                    start=(i == 0),
                                    stop=False,
                                )
                                nc.tensor.matmul(
                                    ps[:],
                                    wB,
                                    B[:, i * W : (i + 1) * W],
                                    start=False,
                                    stop=(i == KC - 1),
                                )
                        h = RPC
                        o = j * W
                        nc.vector.tensor_scalar_max(An[:, o : o + h], ps[:, :h], 0.0)
                        nc.vector.tensor_scalar_min(Bn[:, o : o + h], ps[:, :h], 0.0)
                        nc.vector.tensor_scalar_max(Bn[:, o + h : o + W], ps[:, h:], 0.0)
                        nc.vector.tensor_scalar_min(An[:, o + h : o + W], ps[:, h:], 0.0)
                # bias chain: column-tiled m=1 matvecs, four concurrent in
                # separate 32-column PE groups, accumulating into pbias rows
                # {0,32,64,96}. A-family (rhs mvA) pairs with dbl, B-family
                # (rhs mvB) with dbu.
                for f, rhs_t in enumerate((A, B)):
                    base = (s * 2 + f) * KC
                    for i in range(KC):
                        g = 32 * (i % 4)
                        vcol = hbvt[:, base + i : base + i + 1]
                        if s == 0 and f == 1:
                            nc.tensor.matmul(
                                pbias[g : g + 1, :RPC],
                                vcol,
                                A[:, i * W + RPC : (i + 1) * W],
                                start=False, stop=False, tile_position=(0, g),
                            )
                            nc.tensor.matmul(
                                pbias[g : g + 1, RPC:],
                                vcol,
                                A[:, i * W : i * W + RPC],
                                start=False, stop=False, tile_position=(0, g),
                            )
                        else:
                            nc.tensor.matmul(
                                pbias[g : g + 1, :],
                                vcol,
                                rhs_t[:, i * W : (i + 1) * W],
                                start=(s == 0 and f == 0 and i < 4),
                                stop=False,
                                tile_position=(0, g),
                            )

            # final concretization against the input box, same col-tiled
            # accumulation: mvA pairs with lower_in, mvB with upper_in.
            Af, Bf = mvA[L % 2], mvB[L % 2]
            for f, rhs_t in enumerate((Af, Bf)):
                for i in range(KC):
                    g = 32 * (i % 4)
                    nc.tensor.matmul(
                        pbias[g : g + 1, :],
                        fint[:, f * KC + i : f * KC + i + 1],
                        rhs_t[:, i * W : (i + 1) * W],
                        start=False,
                        stop=(f == 1 and i >= KC - 4),
                        tile_position=(0, g),
                    )

            # res = sum of the four accumulator rows + b (one PSUM operand
            # per DVE instruction)
            acc = bpool.tile([1, W], dt.float32, tag="acc")
            res = bpool.tile([1, W], dt.float32, tag="res")
            nc.vector.tensor_add(acc[:], b2t[:], pbias[0:1, :])
            nc.vector.tensor_add(acc[:], acc[:], pbias[32:33, :])
            nc.vector.tensor_add(acc[:], acc[:], pbias[64:65, :])
            nc.vector.tensor_add(res[:], acc[:], pbias[96:97, :])
            nc.sync.dma_start(out[:], res[:])

    nc.finalize()
    return nc


def _get_nc():
    if "nc" not in _nc_cache:
        _nc_cache["nc"] = _build()
    return _nc_cache["nc"]


def _prep_inputs(A, b, hist_Al, hist_Au, hist_bl, hist_bu, lower_in, upper_in):
    A = np.asarray(A, dtype=np.float32)
    b = np.asarray(b, dtype=np.float32)
    hal = np.asarray(hist_Al, dtype=np.float32)[::-1]
    hau = np.asarray(hist_Au, dtype=np.float32)[::-1]
    hbl = np.asarray(hist_bl, dtype=np.float32)[::-1]
    hbu = np.asarray(hist_bu, dtype=np.float32)[::-1]
    lower_in = np.asarray(lower_in, dtype=np.float32)
    upper_in = np.asarray(upper_in, dtype=np.float32)

    # hist[s, j, t, p, i*P + n] = h_t[s, i*P + p, j*P + n], paired over j
    hist = np.empty([L, KC, 2, P, D], dtype=BF16)
    for t, h in enumerate((hal, hau)):
        hist[:, :, t] = (
            h.reshape(L, KC, P, KC, P).transpose(0, 3, 2, 1, 4).reshape(L, KC, P, D)
        )
    hist = hist.reshape(L, KC // 2, 2, 2, P, D)

    # hbv[p, (s*2+f)*KC + i] = (dbl, dbu)[f][s, i*P + p]
    hbv = (
        np.stack([hbl, hbu], axis=1)  # [L, 2, D]
        .reshape(L * 2 * KC, P)
        .T.astype(BF16)
    )
    hbv = np.ascontiguousarray(hbv)

    # fin[p, t*KC + i]: t=0 lower_in, t=1 upper_in
    fin = (
        np.stack([lower_in.reshape(KC, P), upper_in.reshape(KC, P)], axis=0)
        .transpose(2, 0, 1)
        .reshape(P, 2 * KC)
        .astype(BF16)
    )

    in_maps = []
    for c in range(NCORES):
        At = np.ascontiguousarray(A[c * RPC : (c + 1) * RPC].T)  # [D, RPC]
        at0 = At.reshape(KC, P, RPC).astype(BF16)
        b_blk = b[c * RPC : (c + 1) * RPC]
        b2 = np.concatenate([b_blk, b_blk]).reshape(1, W).astype(np.float32)
        in_maps.append(
            {
                "at0": at0,
                "hist": hist,
                "hbv": hbv,
                "fin": fin,
                "b2": b2,
            }
        )
    return in_maps


def _run(in_maps, trace=False):
    from concourse.bass_utils import run_bass_kernel_spmd

    nc = _get_nc()
    return run_bass_kernel_spmd(
        nc, in_maps, core_ids=list(range(NCORES)), trace=trace
    )


def kernel(A, b, hist_Al, hist_Au, hist_bl, hist_bu, lower_in, upper_in):
    in_maps = _prep_inputs(
        A, b, hist_Al, hist_Au, hist_bl, hist_bu, lower_in, upper_in
    )
    res = _run(in_maps, trace=False)
    lower = np.concatenate([res.results[c]["out"][0, :RPC] for c in range(NCORES)])
    upper = np.concatenate([res.results[c]["out"][0, RPC:] for c in range(NCORES)])
    return lower.astype(np.float32), upper.astype(np.float32)


# revision 12
# speedup vs baseline: 1.0486x; 1.0021x over previous
"""Trainium2 Bass kernel for the affine-transformer backsubstitution chain.

reference semantics (D=2048, L=8):
    Al = Au = A; bl = bu = b
    for s in 0..L-1 (history reversed):
        Al' = relu(Al) @ dAl + min(Al,0) @ dAu
        bl' = relu(Al) @ dbl + min(Al,0) @ dbu + bl
        Au' = relu(Au) @ dAu + min(Au,0) @ dAl
        bu' = relu(Au) @ dbu + min(Au,0) @ dbl + bu
    lower = relu(Al) @ lower_in + min(Al,0) @ upper_in + bl
    upper = relu(Au) @ upper_in + min(Au,0) @ lower_in + bu

Sharding: rows of Al/Au across 8 cores (256 rows each), history replicated.
Per core the state is kept TRANSPOSED ([2048 k-partitions, 256 m-free]) so the
history matrices act directly as matmul weights (out = lhsT.T @ rhs), and the
clamped copies are the state:
    mvA[k] = [ relu(AlT)[k] | min(AuT,0)[k] ]   (pairs with dAl weight tiles)
    mvB[k] = [ min(AlT,0)[k] | relu(AuT)[k] ]   (pairs with dAu weight tiles)
One [128,512] PSUM per output chunk then accumulates both chains at once:
    psum[:, :256] = sum_k dAl[k,n]·relu(AlT) + dAu[k,n]·min(AlT,0) = new AlT
    psum[:, 256:] = sum_k dAl[k,n]·min(AuT,0) + dAu[k,n]·relu(AuT) = new AuT
Compute dtype bf16 (fp32 PSUM accumulation); rel err vs fp32 ≈ 2.5e-3.

The bias chain and the final concretization are m=1 matvecs against the same
state tiles (mvA pairs with dbl/lower_in, mvB with dbu/upper_in). They run as
128x32 column-tiled matmuls — tile_position=(0,32g), g = chunk%4 — so four
stream concurrently in separate column groups of the PE array, and ALL of them
(8 steps x 32 + final 32) accumulate into one PSUM bank on partition rows
{0,32,64,96}; a single DVE pass at the end sums the four rows and adds b.
This costs ~8 serialized matmul slots per step instead of 16 (fp8 DoubleRow)
or 32 (naive), and needs no fp8 shadow state.

PE work per core: 4096 main matmuls x ~220 ns (N=512 stream at 2.4 GHz + NX
dispatch) + ~160 col-tiled matvecs in ~40 4-way groups ≈ 910 µs warm;
runs land ~1.13 ms when the chip drops to its 2.0 GHz P0 power state.
"""

import numpy as np
import ml_dtypes

L = 8
D = 2048
NCORES = 8
RPC = D // NCORES  # 256 rows per core
P = 128
KC = D // P  # 16 partition chunks
W = 2 * RPC  # 512: concatenated moving width

BF16 = ml_dtypes.bfloat16

_nc_cache = {}


def _build():
    from concourse import bacc
    import concourse.tile as tile
    import concourse.mybir as mybir

    dt = mybir.dt
    nc = bacc.Bacc()

    at0 = nc.dram_tensor("at0", [KC, P, RPC], dt.bfloat16, kind="ExternalInput")
    hist = nc.dram_tensor("hist", [L, KC // 2, 2, 2, P, D], dt.bfloat16, kind="ExternalInput")
    # hbv[p, (s*2+f)*KC + i] = (dbl if f==0 else dbu)[s, i*128+p]: per-chunk
    # bias-vector columns used as m=1 stationary weights.
    hbv = nc.dram_tensor("hbv", [P, L * 2 * KC], dt.bfloat16, kind="ExternalInput")
    fin = nc.dram_tensor("fin", [P, 2 * KC], dt.bfloat16, kind="ExternalInput")
    b2 = nc.dram_tensor("b2", [1, W], dt.float32, kind="ExternalInput")
    out = nc.dram_tensor("out", [1, W], dt.float32, kind="ExternalOutput")

    with tile.TileContext(nc) as tc:
        with (
            tc.tile_pool(name="state", bufs=1) as spool,
            tc.tile_pool(name="wts", bufs=4) as wpool,
            tc.tile_pool(name="consts", bufs=1) as cpool,
            tc.tile_pool(name="bias", bufs=1) as bpool,
            tc.tile_pool(name="psum", bufs=7, space="PSUM") as ppool,
            tc.tile_pool(name="psumb", bufs=1, space="PSUM") as pbpool,
        ):
            mvA = [spool.tile([P, KC * W], dt.bfloat16, tag=f"mvA{i}", name=f"mvA{i}") for i in range(2)]
            mvB = [spool.tile([P, KC * W], dt.bfloat16, tag=f"mvB{i}", name=f"mvB{i}") for i in range(2)]
            hbvt = cpool.tile([P, L * 2 * KC], dt.bfloat16, tag="hbvt")
            fint = cpool.tile([P, 2 * KC], dt.bfloat16, tag="fint")
            b2t = bpool.tile([1, W], dt.float32, tag="b2t")

            # One PSUM bank accumulates every m=1 matvec of the kernel (bias
            # chain + final concretization) on partition rows {0,32,64,96}.
            pbias = pbpool.tile([P, W], dt.float32, tag="pb", name="pb")

            # PE warmup: a few cheap matmuls on a zeroed tile bridge the
            # initial DMA window without delaying the first real matmul.
            warm = cpool.tile([P, W], dt.bfloat16, tag="warm")
            nc.vector.memset(warm[:], 0.0)
            pw = ppool.tile([P, W], dt.float32, tag="ps", name="pw")
            for i in range(16):
                nc.tensor.matmul(pw[:, :P], warm[:, :P], warm[:, :P], start=True, stop=True)

            # Startup loads: state chunk-pairs get the sync queue to
            # themselves (each dma_start costs ~0.7 µs of sequencer dispatch,
            # and a pair lands about every 0.7 µs — matching the PE's step-0
            # consumption rate); the first two stripes and the consts go on
            # gpsimd. GpSimd compute is useless here (~4.6 µs per clamp), so
            # the step-0 clamps split relu→ScalarE / min→DVE instead.
            stg = cpool.tile([P, KC, RPC], dt.bfloat16, tag="stg", name="stg")
            stripes = {}
            st00 = wpool.tile([P, 2, 2, D], dt.bfloat16, tag="stripe", name="stripe")
            stripes[(0, 0)] = st00
            h00 = hist[0, 0]
            st01 = wpool.tile([P, 2, 2, D], dt.bfloat16, tag="stripe", name="stripe")
            stripes[(0, 1)] = st01

            def load_pair(q, eng):
                eng.dma_start(
                    stg[:, 2 * q : 2 * (q + 1), :],
                    at0[2 * q : 2 * (q + 1)].rearrange("k p r -> p k r"),
                )

            # Everything group 0 needs (state pairs + all of st00, 3MB)
            # dispatches first, in first-use order; st01 and the consts only
            # after, so their descriptors can't starve the critical loads.
            for g in range(4):
                load_pair(2 * g, nc.sync)
                load_pair(2 * g + 1, nc.gpsimd)
                sl = slice(g * D // 4, (g + 1) * D // 4)
                nc.sync.dma_start(
                    st00[:, :, :, sl], h00[:, :, :, sl].rearrange("jh t p f -> p jh t f")
                )
            nc.gpsimd.dma_start(st01[:], hist[0, 1].rearrange("jh t p f -> p jh t f"))
            nc.gpsimd.dma_start(hbvt[:], hbv[:])
            nc.gpsimd.dma_start(fint[:], fin[:])
            nc.gpsimd.dma_start(b2t[:], b2[:])

            # Step-0 state: Al = Au = A, so only mvA = [relu(AT) | min(AT,0)]
            # is materialized (the B-family reads its halves swapped).
            relu_f = mybir.ActivationFunctionType.Relu
            for i in range(KC):
                o = i * W
                s_i = stg[:, i, :]
                nc.scalar.activation(mvA[0][:, o : o + RPC], s_i, relu_f)
                nc.vector.tensor_scalar_min(mvA[0][:, o + RPC : o + W], s_i, 0.0)

            for s in range(L):
                cur, nxt = s % 2, (s + 1) % 2
                A, B = mvA[cur], mvB[cur]
                An, Bn = mvA[nxt], mvB[nxt]
                for jp in range(KC // 2):
                    if (s, jp) in stripes:
                        stripe = stripes.pop((s, jp))
                    else:
                        stripe = wpool.tile([P, 2, 2, D], dt.bfloat16, tag="stripe", name="stripe")
                        nc.sync.dma_start(
                            stripe[:], hist[s, jp].rearrange("jh t p f -> p jh t f")
                        )
                    for jh in range(2):
                        j = 2 * jp + jh
                        ps = ppool.tile([P, W], dt.float32, tag="ps", name="ps")
                        for i in range(KC):
                            wA = stripe[:, jh, 0, i * P : (i + 1) * P]
                            wB = stripe[:, jh, 1, i * P : (i + 1) * P]
                            if s == 0:
                                # mvB isn't materialized at step 0 (Al = Au):
                                # the B-family reads mvA's halves swapped via
                                # two n=256 matmuls. The i==KC-1 A-matmul is
                                # reordered last to carry the full-width stop.
                                relu_h = A[:, i * W : i * W + RPC]
                                min_h = A[:, i * W + RPC : (i + 1) * W]
                                mms = [
                                    (ps[:], wA, A[:, i * W : (i + 1) * W], i == 0, i == KC - 1),
                                    (ps[:, :RPC], wB, min_h, False, False),
                                    (ps[:, RPC:], wB, relu_h, False, False),
                                ]
                                if i == KC - 1:
                                    mms = mms[1:] + mms[:1]
                                for o_ap, w_ap, r_ap, st, sp in mms:
                                    nc.tensor.matmul(o_ap, w_ap, r_ap, start=st, stop=sp)
                            else:
                                nc.tensor.matmul(
                                    ps[:],
                                    wA,
                                    A[:, i * W : (i + 1) * W],
                                    start=(i == 0),
                                    stop=False,
                                )
                                nc.tensor.matmul(
                                    ps[:],
                                    wB,
                                    B[:, i * W : (i + 1) * W],
                                    start=False,
                                    stop=(i == KC - 1),
                                )
                        h = RPC
                        o = j * W
                        nc.vector.tensor_scalar_max(An[:, o : o + h], ps[:, :h], 0.0)
                        nc.vector.tensor_scalar_min(Bn[:, o : o + h], ps[:, :h], 0.0)
                        nc.vector.tensor_scalar_max(Bn[:, o + h : o + W], ps[:, h:], 0.0)
                        nc.vector.tensor_scalar_min(An[:, o + h : o + W], ps[:, h:], 0.0)
                # bias chain: column-tiled m=1 matvecs, four concurrent in
                # separate 32-column PE groups, accumulating into pbias rows
                # {0,32,64,96}. A-family (rhs mvA) pairs with dbl, B-family
                # (rhs mvB) with dbu.
                for f, rhs_t in enumerate((A, B)):
                    base = (s * 2 + f) * KC
                    for i in range(KC):
                        g = 32 * (i % 4)
                        vcol = hbvt[:, base + i : base + i + 1]
                        if s == 0 and f == 1:
                            nc.tensor.matmul(
                                pbias[g : g + 1, :RPC],
                                vcol,
                                A[:, i * W + RPC : (i + 1) * W],
                                start=False, stop=False, tile_position=(0, g),
                            )
                            nc.tensor.matmul(
                                pbias[g : g + 1, RPC:],
                                vcol,
                                A[:, i * W : i * W + RPC],
                                start=False, stop=False, tile_position=(0, g),
                            )
                        else:
                            nc.tensor.matmul(
                                pbias[g : g + 1, :],
                                vcol,
                                rhs_t[:, i * W : (i + 1) * W],
                                start=(s == 0 and f == 0 and i < 4),
                                stop=False,
                                tile_position=(0, g),
                            )

            # final concretization against the input box, same col-tiled
            # accumulation: mvA pairs with lower_in, mvB with upper_in.
            Af, Bf = mvA[L % 2], mvB[L % 2]
            for f, rhs_t in enumerate((Af, Bf)):
                for i in range(KC):
                    g = 32 * (i % 4)
                    nc.tensor.matmul(
                        pbias[g : g + 1, :],
                        fint[:, f * KC + i : f * KC + i + 1],
                        rhs_t[:, i * W : (i + 1) * W],
                        start=False,
                        stop=(f == 1 and i >= KC - 4),
                        tile_position=(0, g),
                    )

            # res = sum of the four accumulator rows + b (one PSUM operand
            # per DVE instruction)
            acc = bpool.tile([1, W], dt.float32, tag="acc")
            res = bpool.tile([1, W], dt.float32, tag="res")
            nc.vector.tensor_add(acc[:], b2t[:], pbias[0:1, :])
            nc.vector.tensor_add(acc[:], acc[:], pbias[32:33, :])
            nc.vector.tensor_add(acc[:], acc[:], pbias[64:65, :])
            nc.vector.tensor_add(res[:], acc[:], pbias[96:97, :])
            nc.sync.dma_start(out[:], res[:])

    nc.finalize()
    return nc


def _get_nc():
    if "nc" not in _nc_cache:
        _nc_cache["nc"] = _build()
    return _nc_cache["nc"]


def _prep_inputs(A, b, hist_Al, hist_Au, hist_bl, hist_bu, lower_in, upper_in):
    A = np.asarray(A, dtype=np.float32)
    b = np.asarray(b, dtype=np.float32)
    hal = np.asarray(hist_Al, dtype=np.float32)[::-1]
    hau = np.asarray(hist_Au, dtype=np.float32)[::-1]
    hbl = np.asarray(hist_bl, dtype=np.float32)[::-1]
    hbu = np.asarray(hist_bu, dtype=np.float32)[::-1]
    lower_in = np.asarray(lower_in, dtype=np.float32)
    upper_in = np.asarray(upper_in, dtype=np.float32)

    # hist[s, j, t, p, i*P + n] = h_t[s, i*P + p, j*P + n], paired over j
    hist = np.empty([L, KC, 2, P, D], dtype=BF16)
    for t, h in enumerate((hal, hau)):
        hist[:, :, t] = (
            h.reshape(L, KC, P, KC, P).transpose(0, 3, 2, 1, 4).reshape(L, KC, P, D)
        )
    hist = hist.reshape(L, KC // 2, 2, 2, P, D)

    # hbv[p, (s*2+f)*KC + i] = (dbl, dbu)[f][s, i*P + p]
    hbv = (
        np.stack([hbl, hbu], axis=1)  # [L, 2, D]
        .reshape(L * 2 * KC, P)
        .T.astype(BF16)
    )
    hbv = np.ascontiguousarray(hbv)

    # fin[p, t*KC + i]: t=0 lower_in, t=1 upper_in
    fin = (
        np.stack([lower_in.reshape(KC, P), upper_in.reshape(KC, P)], axis=0)
        .transpose(2, 0, 1)
        .reshape(P, 2 * KC)
        .astype(BF16)
    )

    in_maps = []
    for c in range(NCORES):
        At = np.ascontiguousarray(A[c * RPC : (c + 1) * RPC].T)  # [D, RPC]
        at0 = At.reshape(KC, P, RPC).astype(BF16)
        b_blk = b[c * RPC : (c + 1) * RPC]
        b2 = np.concatenate([b_blk, b_blk]).reshape(1, W).astype(np.float32)
        in_maps.append(
            {
                "at0": at0,
                "hist": hist,
                "hbv": hbv,
                "fin": fin,
                "b2": b2,
            }
        )
    return in_maps


def _run(in_maps, trace=False):
    from concourse.bass_utils import run_bass_kernel_spmd

    nc = _get_nc()
    return run_bass_kernel_spmd(
        nc, in_maps, core_ids=list(range(NCORES)), trace=trace
    )


def kernel(A, b, hist_Al, hist_Au, hist_bl, hist_bu, lower_in, upper_in):
    in_maps = _prep_inputs(
        A, b, hist_Al, hist_Au, hist_bl, hist_bu, lower_in, upper_in
    )
    res = _run(in_maps, trace=False)
    lower = np.concatenate([res.results[c]["out"][0, :RPC] for c in range(NCORES)])
    upper = np.concatenate([res.results[c]["out"][0, RPC:] for c in range(NCORES)])
    return lower.astype(np.float32), upper.astype(np.float32)


# revision 13
# speedup vs baseline: 1.2524x; 1.1944x over previous
"""Trainium2 Bass kernel for the affine-transformer backsubstitution chain.

reference semantics (D=2048, L=8):
    Al = Au = A; bl = bu = b
    for s in 0..L-1 (history reversed):
        Al' = relu(Al) @ dAl + min(Al,0) @ dAu
        bl' = relu(Al) @ dbl + min(Al,0) @ dbu + bl
        Au' = relu(Au) @ dAu + min(Au,0) @ dAl
        bu' = relu(Au) @ dbu + min(Au,0) @ dbl + bu
    lower = relu(Al) @ lower_in + min(Al,0) @ upper_in + bl
    upper = relu(Au) @ upper_in + min(Au,0) @ lower_in + bu

Sharding: rows of Al/Au across 8 cores (256 rows each), history replicated.
Per core the state is kept TRANSPOSED ([2048 k-partitions, 256 m-free]) so the
history matrices act directly as matmul weights (out = lhsT.T @ rhs), and the
clamped copies are the state:
    mvA[k] = [ relu(AlT)[k] | min(AuT,0)[k] ]   (pairs with dAl weight tiles)
    mvB[k] = [ min(AlT,0)[k] | relu(AuT)[k] ]   (pairs with dAu weight tiles)
One [128,512] PSUM per output chunk then accumulates both chains at once:
    psum[:, :256] = sum_k dAl[k,n]·relu(AlT) + dAu[k,n]·min(AlT,0) = new AlT
    psum[:, 256:] = sum_k dAl[k,n]·min(AuT,0) + dAu[k,n]·relu(AuT) = new AuT
Compute dtype bf16 (fp32 PSUM accumulation); rel err vs fp32 ≈ 2.5e-3.

The bias chain and the final concretization are m=1 matvecs against the same
state tiles (mvA pairs with dbl/lower_in, mvB with dbu/upper_in). They run as
128x32 column-tiled matmuls — tile_position=(0,32g), g = chunk%4 — so four
stream concurrently in separate column groups of the PE array, and ALL of them
(8 steps x 32 + final 32) accumulate into one PSUM bank on partition rows
{0,32,64,96}; a single DVE pass at the end sums the four rows and adds b.
This costs ~8 serialized matmul slots per step instead of 16 (fp8 DoubleRow)
or 32 (naive), and needs no fp8 shadow state.

PE work per core: 4096 main matmuls x ~220 ns (N=512 stream at 2.4 GHz + NX
dispatch) + ~160 col-tiled matvecs in ~40 4-way groups ≈ 910 µs warm;
runs land ~1.13 ms when the chip drops to its 2.0 GHz P0 power state.
"""

import numpy as np
import ml_dtypes

L = 8
D = 2048
NCORES = 8
RPC = D // NCORES  # 256 rows per core
P = 128
KC = D // P  # 16 partition chunks
W = 2 * RPC  # 512: concatenated moving width

BF16 = ml_dtypes.bfloat16

_nc_cache = {}


def _build():
    from concourse import bacc
    import concourse.tile as tile
    import concourse.mybir as mybir

    dt = mybir.dt
    nc = bacc.Bacc()

    at0 = nc.dram_tensor("at0", [KC, P, RPC], dt.bfloat16, kind="ExternalInput")
    hist = nc.dram_tensor("hist", [L, KC // 2, 2, 2, P, D], dt.bfloat16, kind="ExternalInput")
    # hbv[p, (s*2+f)*KC + i] = (dbl if f==0 else dbu)[s, i*128+p]: per-chunk
    # bias-vector columns used as m=1 stationary weights.
    hbv = nc.dram_tensor("hbv", [P, L * 2 * KC], dt.bfloat16, kind="ExternalInput")
    fin = nc.dram_tensor("fin", [P, 2 * KC], dt.bfloat16, kind="ExternalInput")
    b2 = nc.dram_tensor("b2", [1, W], dt.float32, kind="ExternalInput")
    out = nc.dram_tensor("out", [1, W], dt.float32, kind="ExternalOutput")

    with tile.TileContext(nc) as tc:
        with (
            tc.tile_pool(name="state", bufs=1) as spool,
            tc.tile_pool(name="wts", bufs=4) as wpool,
            tc.tile_pool(name="consts", bufs=1) as cpool,
            tc.tile_pool(name="bias", bufs=1) as bpool,
            tc.tile_pool(name="psum", bufs=7, space="PSUM") as ppool,
            tc.tile_pool(name="psumb", bufs=1, space="PSUM") as pbpool,
        ):
            mvA = [spool.tile([P, KC * W], dt.bfloat16, tag=f"mvA{i}", name=f"mvA{i}") for i in range(2)]
            mvB = [spool.tile([P, KC * W], dt.bfloat16, tag=f"mvB{i}", name=f"mvB{i}") for i in range(2)]
            hbvt = cpool.tile([P, L * 2 * KC], dt.bfloat16, tag="hbvt")
            fint = cpool.tile([P, 2 * KC], dt.bfloat16, tag="fint")
            b2t = bpool.tile([1, W], dt.float32, tag="b2t")

            # One PSUM bank accumulates every m=1 matvec of the kernel (bias
            # chain + final concretization) on partition rows {0,32,64,96}.
            pbias = pbpool.tile([P, W], dt.float32, tag="pb", name="pb")

            # PE warmup: a few cheap matmuls on a zeroed tile bridge the
            # initial DMA window without delaying the first real matmul.
            warm = cpool.tile([P, W], dt.bfloat16, tag="warm")
            nc.vector.memset(warm[:], 0.0)
            pw = ppool.tile([P, W], dt.float32, tag="ps", name="pw")
            for i in range(16):
                nc.tensor.matmul(pw[:, :P], warm[:, :P], warm[:, :P], start=True, stop=True)

            # Startup loads: state chunk-pairs get the sync queue to
            # themselves (each dma_start costs ~0.7 µs of sequencer dispatch,
            # and a pair lands about every 0.7 µs — matching the PE's step-0
            # consumption rate); the first two stripes and the consts go on
            # gpsimd. GpSimd compute is useless here (~4.6 µs per clamp), so
            # the step-0 clamps split relu→ScalarE / min→DVE instead.
            stg = cpool.tile([P, KC, RPC], dt.bfloat16, tag="stg", name="stg")
            stripes = {}
            st00 = wpool.tile([P, 2, 2, D], dt.bfloat16, tag="stripe", name="stripe")
            stripes[(0, 0)] = st00
            h00 = hist[0, 0]
            st01 = wpool.tile([P, 2, 2, D], dt.bfloat16, tag="stripe", name="stripe")
            stripes[(0, 1)] = st01

            def load_pair(q, eng):
                eng.dma_start(
                    stg[:, 2 * q : 2 * (q + 1), :],
                    at0[2 * q : 2 * (q + 1)].rearrange("k p r -> p k r"),
                )

            # Everything group 0 needs (state pairs + all of st00, 3MB)
            # dispatches first, in first-use order; st01 and the consts only
            # after, so their descriptors can't starve the critical loads.
            for g in range(4):
                load_pair(2 * g, nc.sync)
                load_pair(2 * g + 1, nc.gpsimd)
                sl = slice(g * D // 4, (g + 1) * D // 4)
                nc.sync.dma_start(
                    st00[:, :, :, sl], h00[:, :, :, sl].rearrange("jh t p f -> p jh t f")
                )
            nc.gpsimd.dma_start(st01[:], hist[0, 1].rearrange("jh t p f -> p jh t f"))
            nc.gpsimd.dma_start(hbvt[:], hbv[:])
            nc.gpsimd.dma_start(fint[:], fin[:])
            nc.gpsimd.dma_start(b2t[:], b2[:])

            # Step-0 state: Al = Au = A, so only mvA = [relu(AT) | min(AT,0)]
            # is materialized. ScalarE (relu, ~540ns/op) and DVE (min,
            # ~220ns/op) split the chain so both finish together.
            relu_f = mybir.ActivationFunctionType.Relu
            for i in range(KC):
                o = i * W
                s_i = stg[:, i, :]
                if i < 9:
                    nc.scalar.activation(mvA[0][:, o : o + RPC], s_i, relu_f)
                else:
                    nc.vector.tensor_scalar_max(mvA[0][:, o : o + RPC], s_i, 0.0)
                nc.vector.tensor_scalar_min(mvA[0][:, o + RPC : o + W], s_i, 0.0)

            for s in range(L):
                cur, nxt = s % 2, (s + 1) % 2
                A, B = mvA[cur], mvB[cur]
                An, Bn = mvA[nxt], mvB[nxt]
                for jp in range(KC // 2):
                    if (s, jp) in stripes:
                        stripe = stripes.pop((s, jp))
                    else:
                        stripe = wpool.tile([P, 2, 2, D], dt.bfloat16, tag="stripe", name="stripe")
                        nc.sync.dma_start(
                            stripe[:], hist[s, jp].rearrange("jh t p f -> p jh t f")
                        )
                    for jh in range(2):
                        j = 2 * jp + jh
                        ps = ppool.tile([P, W], dt.float32, tag="ps", name="ps")
                        for i in range(KC):
                            wA = stripe[:, jh, 0, i * P : (i + 1) * P]
                            wB = stripe[:, jh, 1, i * P : (i + 1) * P]
                            if s == 0:
                                # mvB isn't materialized at step 0 (Al = Au):
                                # the B-family reads mvA's halves swapped via
                                # two n=256 matmuls. The i==KC-1 A-matmul is
                                # reordered last to carry the full-width stop.
                                relu_h = A[:, i * W : i * W + RPC]
                                min_h = A[:, i * W + RPC : (i + 1) * W]
                                mms = [
                                    (ps[:], wA, A[:, i * W : (i + 1) * W], i == 0, i == KC - 1),
                                    (ps[:, :RPC], wB, min_h, False, False),
                                    (ps[:, RPC:], wB, relu_h, False, False),
                                ]
                                if i == KC - 1:
                                    mms = mms[1:] + mms[:1]
                                for o_ap, w_ap, r_ap, st, sp in mms:
                                    nc.tensor.matmul(o_ap, w_ap, r_ap, start=st, stop=sp)
                            else:
                                nc.tensor.matmul(
                                    ps[:],
                                    wA,
                                    A[:, i * W : (i + 1) * W],
                                    start=(i == 0),
                                    stop=False,
                                )
                                nc.tensor.matmul(
                                    ps[:],
                                    wB,
                                    B[:, i * W : (i + 1) * W],
                                    start=False,
                                    stop=(i == KC - 1),
                                )
                        h = RPC
                        o = j * W
                        nc.vector.tensor_scalar_max(An[:, o : o + h], ps[:, :h], 0.0)
                        nc.vector.tensor_scalar_min(Bn[:, o : o + h], ps[:, :h], 0.0)
                        nc.vector.tensor_scalar_max(Bn[:, o + h : o + W], ps[:, h:], 0.0)
                        nc.vector.tensor_scalar_min(An[:, o + h : o + W], ps[:, h:], 0.0)
                # bias chain: column-tiled m=1 matvecs, four concurrent in
                # separate 32-column PE groups, accumulating into pbias rows
                # {0,32,64,96}. A-family (rhs mvA) pairs with dbl, B-family
                # (rhs mvB) with dbu.
                for f, rhs_t in enumerate((A, B)):
                    base = (s * 2 + f) * KC
                    for i in range(KC):
                        g = 32 * (i % 4)
                        vcol = hbvt[:, base + i : base + i + 1]
                        if s == 0 and f == 1:
                            nc.tensor.matmul(
                                pbias[g : g + 1, :RPC],
                                vcol,
                                A[:, i * W + RPC : (i + 1) * W],
                                start=False, stop=False, tile_position=(0, g),
                            )
                            nc.tensor.matmul(
                                pbias[g : g + 1, RPC:],
                                vcol,
                                A[:, i * W : i * W + RPC],
                                start=False, stop=False, tile_position=(0, g),
                            )
                        else:
                            nc.tensor.matmul(
                                pbias[g : g + 1, :],
                                vcol,
                                rhs_t[:, i * W : (i + 1) * W],
                                start=(s == 0 and f == 0 and i < 4),
                                stop=False,
                                tile_position=(0, g),
                            )

            # final concretization against the input box, same col-tiled
            # accumulation: mvA pairs with lower_in, mvB with upper_in.
            Af, Bf = mvA[L % 2], mvB[L % 2]
            for f, rhs_t in enumerate((Af, Bf)):
                for i in range(KC):
                    g = 32 * (i % 4)
                    nc.tensor.matmul(
                        pbias[g : g + 1, :],
                        fint[:, f * KC + i : f * KC + i + 1],
                        rhs_t[:, i * W : (i + 1) * W],
                        start=False,
                        stop=(f == 1 and i >= KC - 4),
                        tile_position=(0, g),
                    )

            # res = sum of the four accumulator rows + b (one PSUM operand
            # per DVE instruction)
            acc = bpool.tile([1, W], dt.float32, tag="acc")
            res = bpool.tile([1, W], dt.float32, tag="res")
            nc.vector.tensor_add(acc[:], b2t[:], pbias[0:1, :])
            nc.vector.tensor_add(acc[:], acc[:], pbias[32:33, :])
            nc.vector.tensor_add(acc[:], acc[:], pbias[64:65, :])
            nc.vector.tensor_add(res[:], acc[:], pbias[96:97, :])
            nc.sync.dma_start(out[:], res[:])

    nc.finalize()
    return nc


def _get_nc():
    if "nc" not in _nc_cache:
        _nc_cache["nc"] = _build()
    return _nc_cache["nc"]


def _prep_inputs(A, b, hist_Al, hist_Au, hist_bl, hist_bu, lower_in, upper_in):
    A = np.asarray(A, dtype=np.float32)
    b = np.asarray(b, dtype=np.float32)
    hal = np.asarray(hist_Al, dtype=np.float32)[::-1]
    hau = np.asarray(hist_Au, dtype=np.float32)[::-1]
    hbl = np.asarray(hist_bl, dtype=np.float32)[::-1]
    hbu = np.asarray(hist_bu, dtype=np.float32)[::-1]
    lower_in = np.asarray(lower_in, dtype=np.float32)
    upper_in = np.asarray(upper_in, dtype=np.float32)

    # hist[s, j, t, p, i*P + n] = h_t[s, i*P + p, j*P + n], paired over j
    hist = np.empty([L, KC, 2, P, D], dtype=BF16)
    for t, h in enumerate((hal, hau)):
        hist[:, :, t] = (
            h.reshape(L, KC, P, KC, P).transpose(0, 3, 2, 1, 4).reshape(L, KC, P, D)
        )
    hist = hist.reshape(L, KC // 2, 2, 2, P, D)

    # hbv[p, (s*2+f)*KC + i] = (dbl, dbu)[f][s, i*P + p]
    hbv = (
        np.stack([hbl, hbu], axis=1)  # [L, 2, D]
        .reshape(L * 2 * KC, P)
        .T.astype(BF16)
    )
    hbv = np.ascontiguousarray(hbv)

    # fin[p, t*KC + i]: t=0 lower_in, t=1 upper_in
    fin = (
        np.stack([lower_in.reshape(KC, P), upper_in.reshape(KC, P)], axis=0)
        .transpose(2, 0, 1)
        .reshape(P, 2 * KC)
        .astype(BF16)
    )

    in_maps = []
    for c in range(NCORES):
        At = np.ascontiguousarray(A[c * RPC : (c + 1) * RPC].T)  # [D, RPC]
        at0 = At.reshape(KC, P, RPC).astype(BF16)
        b_blk = b[c * RPC : (c + 1) * RPC]
        b2 = np.concatenate([b_blk, b_blk]).reshape(1, W).astype(np.float32)
        in_maps.append(
            {
                "at0": at0,
                "hist": hist,
                "hbv": hbv,
                "fin": fin,
                "b2": b2,
            }
        )
    return in_maps


def _run(in_maps, trace=False):
    from concourse.bass_utils import run_bass_kernel_spmd

    nc = _get_nc()
    return run_bass_kernel_spmd(
        nc, in_maps, core_ids=list(range(NCORES)), trace=trace
    )


def kernel(A, b, hist_Al, hist_Au, hist_bl, hist_bu, lower_in, upper_in):
    in_maps = _prep_inputs(
        A, b, hist_Al, hist_Au, hist_bl, hist_bu, lower_in, upper_in
    )
    res = _run(in_maps, trace=False)
    lower = np.concatenate([res.results[c]["out"][0, :RPC] for c in range(NCORES)])
    upper = np.concatenate([res.results[c]["out"][0, RPC:] for c in range(NCORES)])
    return lower.astype(np.float32), upper.astype(np.float32)


# revision 14
# speedup vs baseline: 1.2542x; 1.0015x over previous
"""Trainium2 Bass kernel for the affine-transformer backsubstitution chain.

reference semantics (D=2048, L=8):
    Al = Au = A; bl = bu = b
    for s in 0..L-1 (history reversed):
        Al' = relu(Al) @ dAl + min(Al,0) @ dAu
        bl' = relu(Al) @ dbl + min(Al,0) @ dbu + bl
        Au' = relu(Au) @ dAu + min(Au,0) @ dAl
        bu' = relu(Au) @ dbu + min(Au,0) @ dbl + bu
    lower = relu(Al) @ lower_in + min(Al,0) @ upper_in + bl
    upper = relu(Au) @ upper_in + min(Au,0) @ lower_in + bu

Sharding: rows of Al/Au across 8 cores (256 rows each), history replicated.
Per core the state is kept TRANSPOSED ([2048 k-partitions, 256 m-free]) so the
history matrices act directly as matmul weights (out = lhsT.T @ rhs), and the
clamped copies are the state:
    mvA[k] = [ relu(AlT)[k] | min(AuT,0)[k] ]   (pairs with dAl weight tiles)
    mvB[k] = [ min(AlT,0)[k] | relu(AuT)[k] ]   (pairs with dAu weight tiles)
One [128,512] PSUM per output chunk then accumulates both chains at once:
    psum[:, :256] = sum_k dAl[k,n]·relu(AlT) + dAu[k,n]·min(AlT,0) = new AlT
    psum[:, 256:] = sum_k dAl[k,n]·min(AuT,0) + dAu[k,n]·relu(AuT) = new AuT
Compute dtype bf16 (fp32 PSUM accumulation); rel err vs fp32 ≈ 2.5e-3.

The bias chain and the final concretization are m=1 matvecs against the same
state tiles (mvA pairs with dbl/lower_in, mvB with dbu/upper_in). They run as
128x32 column-tiled matmuls — tile_position=(0,32g), g = chunk%4 — so four
stream concurrently in separate column groups of the PE array, and ALL of them
(8 steps x 32 + final 32) accumulate into one PSUM bank on partition rows
{0,32,64,96}; a single DVE pass at the end sums the four rows and adds b.
This costs ~8 serialized matmul slots per step instead of 16 (fp8 DoubleRow)
or 32 (naive), and needs no fp8 shadow state.

PE work per core: 4096 main matmuls x ~220 ns (N=512 stream at 2.4 GHz + NX
dispatch) + ~160 col-tiled matvecs in ~40 4-way groups ≈ 910 µs warm;
runs land ~1.13 ms when the chip drops to its 2.0 GHz P0 power state.
"""

import numpy as np
import ml_dtypes

L = 8
D = 2048
NCORES = 8
RPC = D // NCORES  # 256 rows per core
P = 128
KC = D // P  # 16 partition chunks
W = 2 * RPC  # 512: concatenated moving width

BF16 = ml_dtypes.bfloat16

_nc_cache = {}


def _build():
    from concourse import bacc
    import concourse.tile as tile
    import concourse.mybir as mybir

    dt = mybir.dt
    nc = bacc.Bacc()

    at0 = nc.dram_tensor("at0", [KC, P, RPC], dt.bfloat16, kind="ExternalInput")
    hist = nc.dram_tensor("hist", [L, KC // 2, 2, 2, P, D], dt.bfloat16, kind="ExternalInput")
    # hbv[p, (s*2+f)*KC + i] = (dbl if f==0 else dbu)[s, i*128+p]: per-chunk
    # bias-vector columns used as m=1 stationary weights.
    hbv = nc.dram_tensor("hbv", [P, L * 2 * KC], dt.bfloat16, kind="ExternalInput")
    fin = nc.dram_tensor("fin", [P, 2 * KC], dt.bfloat16, kind="ExternalInput")
    b2 = nc.dram_tensor("b2", [1, W], dt.float32, kind="ExternalInput")
    out = nc.dram_tensor("out", [1, W], dt.float32, kind="ExternalOutput")

    with tile.TileContext(nc) as tc:
        with (
            tc.tile_pool(name="state", bufs=1) as spool,
            tc.tile_pool(name="wts", bufs=4) as wpool,
            tc.tile_pool(name="consts", bufs=1) as cpool,
            tc.tile_pool(name="bias", bufs=1) as bpool,
            tc.tile_pool(name="psum", bufs=7, space="PSUM") as ppool,
            tc.tile_pool(name="psumb", bufs=1, space="PSUM") as pbpool,
        ):
            mvA = [spool.tile([P, KC * W], dt.bfloat16, tag=f"mvA{i}", name=f"mvA{i}") for i in range(2)]
            mvB = [spool.tile([P, KC * W], dt.bfloat16, tag=f"mvB{i}", name=f"mvB{i}") for i in range(2)]
            hbvt = cpool.tile([P, L * 2 * KC], dt.bfloat16, tag="hbvt")
            fint = cpool.tile([P, 2 * KC], dt.bfloat16, tag="fint")
            b2t = bpool.tile([1, W], dt.float32, tag="b2t")

            # One PSUM bank accumulates every m=1 matvec of the kernel (bias
            # chain + final concretization) on partition rows {0,32,64,96}.
            pbias = pbpool.tile([P, W], dt.float32, tag="pb", name="pb")

            # PE warmup: a few cheap matmuls on a zeroed tile bridge the
            # initial DMA window without delaying the first real matmul.
            warm = cpool.tile([P, W], dt.bfloat16, tag="warm")
            nc.vector.memset(warm[:], 0.0)
            pw = ppool.tile([P, W], dt.float32, tag="ps", name="pw")
            for i in range(16):
                nc.tensor.matmul(pw[:, :P], warm[:, :P], warm[:, :P], start=True, stop=True)

            # Startup loads: state chunk-pairs get the sync queue to
            # themselves (each dma_start costs ~0.7 µs of sequencer dispatch,
            # and a pair lands about every 0.7 µs — matching the PE's step-0
            # consumption rate); the first two stripes and the consts go on
            # gpsimd. GpSimd compute is useless here (~4.6 µs per clamp), so
            # the step-0 clamps split relu→ScalarE / min→DVE instead.
            stg = cpool.tile([P, KC, RPC], dt.bfloat16, tag="stg", name="stg")
            stripes = {}
            st00 = wpool.tile([P, 2, 2, D], dt.bfloat16, tag="stripe", name="stripe")
            stripes[(0, 0)] = st00
            h00 = hist[0, 0]
            st01 = wpool.tile([P, 2, 2, D], dt.bfloat16, tag="stripe", name="stripe")
            stripes[(0, 1)] = st01

            def load_pair(q, eng):
                eng.dma_start(
                    stg[:, 2 * q : 2 * (q + 1), :],
                    at0[2 * q : 2 * (q + 1)].rearrange("k p r -> p k r"),
                )

            # Everything group 0 needs (state pairs + st00's jh=0 half, 2MB)
            # dispatches first, in first-use order; jh=1, st01 and the consts
            # follow, so their descriptors can't starve the critical loads.
            def load_st00(jh, half):
                sl = slice(half * D // 2, (half + 1) * D // 2)
                nc.sync.dma_start(
                    st00[:, jh, :, sl],
                    h00[jh, :, :, sl].rearrange("t p f -> p t f"),
                )

            load_pair(0, nc.sync)
            load_pair(1, nc.gpsimd)
            load_st00(0, 0)
            load_pair(2, nc.sync)
            load_pair(3, nc.gpsimd)
            load_pair(4, nc.sync)
            load_st00(0, 1)
            load_pair(5, nc.gpsimd)
            load_pair(6, nc.sync)
            load_pair(7, nc.gpsimd)
            load_st00(1, 0)
            load_st00(1, 1)
            nc.gpsimd.dma_start(st01[:], hist[0, 1].rearrange("jh t p f -> p jh t f"))
            nc.gpsimd.dma_start(hbvt[:], hbv[:])
            nc.gpsimd.dma_start(fint[:], fin[:])
            nc.gpsimd.dma_start(b2t[:], b2[:])

            # Step-0 state: Al = Au = A, so only mvA = [relu(AT) | min(AT,0)]
            # is materialized. ScalarE (relu, ~540ns/op) and DVE (min,
            # ~220ns/op) split the chain so both finish together.
            relu_f = mybir.ActivationFunctionType.Relu
            for i in range(KC):
                o = i * W
                s_i = stg[:, i, :]
                if i < 9:
                    nc.scalar.activation(mvA[0][:, o : o + RPC], s_i, relu_f)
                else:
                    nc.vector.tensor_scalar_max(mvA[0][:, o : o + RPC], s_i, 0.0)
                nc.vector.tensor_scalar_min(mvA[0][:, o + RPC : o + W], s_i, 0.0)

            for s in range(L):
                cur, nxt = s % 2, (s + 1) % 2
                A, B = mvA[cur], mvB[cur]
                An, Bn = mvA[nxt], mvB[nxt]
                for jp in range(KC // 2):
                    if (s, jp) in stripes:
                        stripe = stripes.pop((s, jp))
                    else:
                        stripe = wpool.tile([P, 2, 2, D], dt.bfloat16, tag="stripe", name="stripe")
                        nc.sync.dma_start(
                            stripe[:], hist[s, jp].rearrange("jh t p f -> p jh t f")
                        )
                    for jh in range(2):
                        j = 2 * jp + jh
                        ps = ppool.tile([P, W], dt.float32, tag="ps", name="ps")
                        for i in range(KC):
                            wA = stripe[:, jh, 0, i * P : (i + 1) * P]
                            wB = stripe[:, jh, 1, i * P : (i + 1) * P]
                            if s == 0:
                                # mvB isn't materialized at step 0 (Al = Au):
                                # the B-family reads mvA's halves swapped via
                                # two n=256 matmuls. The i==KC-1 A-matmul is
                                # reordered last to carry the full-width stop.
                                relu_h = A[:, i * W : i * W + RPC]
                                min_h = A[:, i * W + RPC : (i + 1) * W]
                                mms = [
                                    (ps[:], wA, A[:, i * W : (i + 1) * W], i == 0, i == KC - 1),
                                    (ps[:, :RPC], wB, min_h, False, False),
                                    (ps[:, RPC:], wB, relu_h, False, False),
                                ]
                                if i == KC - 1:
                                    mms = mms[1:] + mms[:1]
                                for o_ap, w_ap, r_ap, st, sp in mms:
                                    nc.tensor.matmul(o_ap, w_ap, r_ap, start=st, stop=sp)
                            else:
                                nc.tensor.matmul(
                                    ps[:],
                                    wA,
                                    A[:, i * W : (i + 1) * W],
                                    start=(i == 0),
                                    stop=False,
                                )
                                nc.tensor.matmul(
                                    ps[:],
                                    wB,
                                    B[:, i * W : (i + 1) * W],
                                    start=False,
                                    stop=(i == KC - 1),
                                )
                        h = RPC
                        o = j * W
                        nc.vector.tensor_scalar_max(An[:, o : o + h], ps[:, :h], 0.0)
                        nc.vector.tensor_scalar_min(Bn[:, o : o + h], ps[:, :h], 0.0)
                        nc.vector.tensor_scalar_max(Bn[:, o + h : o + W], ps[:, h:], 0.0)
                        nc.vector.tensor_scalar_min(An[:, o + h : o + W], ps[:, h:], 0.0)
                # bias chain: column-tiled m=1 matvecs, four concurrent in
                # separate 32-column PE groups, accumulating into pbias rows
                # {0,32,64,96}. A-family (rhs mvA) pairs with dbl, B-family
                # (rhs mvB) with dbu.
                for f, rhs_t in enumerate((A, B)):
                    base = (s * 2 + f) * KC
                    for i in range(KC):
                        g = 32 * (i % 4)
                        vcol = hbvt[:, base + i : base + i + 1]
                        if s == 0 and f == 1:
                            nc.tensor.matmul(
                                pbias[g : g + 1, :RPC],
                                vcol,
                                A[:, i * W + RPC : (i + 1) * W],
                                start=False, stop=False, tile_position=(0, g),
                            )
                            nc.tensor.matmul(
                                pbias[g : g + 1, RPC:],
                                vcol,
                                A[:, i * W : i * W + RPC],
                                start=False, stop=False, tile_position=(0, g),
                            )
                        else:
                            nc.tensor.matmul(
                                pbias[g : g + 1, :],
                                vcol,
                                rhs_t[:, i * W : (i + 1) * W],
                                start=(s == 0 and f == 0 and i < 4),
                                stop=False,
                                tile_position=(0, g),
                            )

            # final concretization against the input box, same col-tiled
            # accumulation: mvA pairs with lower_in, mvB with upper_in.
            Af, Bf = mvA[L % 2], mvB[L % 2]
            for f, rhs_t in enumerate((Af, Bf)):
                for i in range(KC):
                    g = 32 * (i % 4)
                    nc.tensor.matmul(
                        pbias[g : g + 1, :],
                        fint[:, f * KC + i : f * KC + i + 1],
                        rhs_t[:, i * W : (i + 1) * W],
                        start=False,
                        stop=(f == 1 and i >= KC - 4),
                        tile_position=(0, g),
                    )

            # res = sum of the four accumulator rows + b (one PSUM operand
            # per DVE instruction)
            acc = bpool.tile([1, W], dt.float32, tag="acc")
            res = bpool.tile([1, W], dt.float32, tag="res")
            nc.vector.tensor_add(acc[:], b2t[:], pbias[0:1, :])
            nc.vector.tensor_add(acc[:], acc[:], pbias[32:33, :])
            nc.vector.tensor_add(acc[:], acc[:], pbias[64:65, :])
            nc.vector.tensor_add(res[:], acc[:], pbias[96:97, :])
            nc.sync.dma_start(out[:], res[:])

    nc.finalize()
    return nc


def _get_nc():
    if "nc" not in _nc_cache:
        _nc_cache["nc"] = _build()
    return _nc_cache["nc"]


def _prep_inputs(A, b, hist_Al, hist_Au, hist_bl, hist_bu, lower_in, upper_in):
    A = np.asarray(A, dtype=np.float32)
    b = np.asarray(b, dtype=np.float32)
    hal = np.asarray(hist_Al, dtype=np.float32)[::-1]
    hau = np.asarray(hist_Au, dtype=np.float32)[::-1]
    hbl = np.asarray(hist_bl, dtype=np.float32)[::-1]
    hbu = np.asarray(hist_bu, dtype=np.float32)[::-1]
    lower_in = np.asarray(lower_in, dtype=np.float32)
    upper_in = np.asarray(upper_in, dtype=np.float32)

    # hist[s, j, t, p, i*P + n] = h_t[s, i*P + p, j*P + n], paired over j
    hist = np.empty([L, KC, 2, P, D], dtype=BF16)
    for t, h in enumerate((hal, hau)):
        hist[:, :, t] = (
            h.reshape(L, KC, P, KC, P).transpose(0, 3, 2, 1, 4).reshape(L, KC, P, D)
        )
    hist = hist.reshape(L, KC // 2, 2, 2, P, D)

    # hbv[p, (s*2+f)*KC + i] = (dbl, dbu)[f][s, i*P + p]
    hbv = (
        np.stack([hbl, hbu], axis=1)  # [L, 2, D]
        .reshape(L * 2 * KC, P)
        .T.astype(BF16)
    )
    hbv = np.ascontiguousarray(hbv)

    # fin[p, t*KC + i]: t=0 lower_in, t=1 upper_in
    fin = (
        np.stack([lower_in.reshape(KC, P), upper_in.reshape(KC, P)], axis=0)
        .transpose(2, 0, 1)
        .reshape(P, 2 * KC)
        .astype(BF16)
    )

    in_maps = []
    for c in range(NCORES):
        At = np.ascontiguousarray(A[c * RPC : (c + 1) * RPC].T)  # [D, RPC]
        at0 = At.reshape(KC, P, RPC).astype(BF16)
        b_blk = b[c * RPC : (c + 1) * RPC]
        b2 = np.concatenate([b_blk, b_blk]).reshape(1, W).astype(np.float32)
        in_maps.append(
            {
                "at0": at0,
                "hist": hist,
                "hbv": hbv,
                "fin": fin,
                "b2": b2,
            }
        )
    return in_maps


def _run(in_maps, trace=False):
    from concourse.bass_utils import run_bass_kernel_spmd

    nc = _get_nc()
    return run_bass_kernel_spmd(
        nc, in_maps, core_ids=list(range(NCORES)), trace=trace
    )


def kernel(A, b, hist_Al, hist_Au, hist_bl, hist_bu, lower_in, upper_in):
    in_maps = _prep_inputs(
        A, b, hist_Al, hist_Au, hist_bl, hist_bu, lower_in, upper_in
    )
    res = _run(in_maps, trace=False)
    lower = np.concatenate([res.results[c]["out"][0, :RPC] for c in range(NCORES)])
    upper = np.concatenate([res.results[c]["out"][0, RPC:] for c in range(NCORES)])
    return lower.astype(np.float32), upper.astype(np.float32)


# revision 16
# speedup vs baseline: 1.2576x; 1.0027x over previous
"""Trainium2 Bass kernel for the affine-transformer backsubstitution chain.

reference semantics (D=2048, L=8):
    Al = Au = A; bl = bu = b
    for s in 0..L-1 (history reversed):
        Al' = relu(Al) @ dAl + min(Al,0) @ dAu
        bl' = relu(Al) @ dbl + min(Al,0) @ dbu + bl
        Au' = relu(Au) @ dAu + min(Au,0) @ dAl
        bu' = relu(Au) @ dbu + min(Au,0) @ dbl + bu
    lower = relu(Al) @ lower_in + min(Al,0) @ upper_in + bl
    upper = relu(Au) @ upper_in + min(Au,0) @ lower_in + bu

Sharding: rows of Al/Au across 8 cores (256 rows each), history replicated.
Per core the state is kept TRANSPOSED ([2048 k-partitions, 256 m-free]) so the
history matrices act directly as matmul weights (out = lhsT.T @ rhs), and the
clamped copies are the state:
    mvA[k] = [ relu(AlT)[k] | min(AuT,0)[k] ]   (pairs with dAl weight tiles)
    mvB[k] = [ min(AlT,0)[k] | relu(AuT)[k] ]   (pairs with dAu weight tiles)
One [128,512] PSUM per output chunk then accumulates both chains at once:
    psum[:, :256] = sum_k dAl[k,n]·relu(AlT) + dAu[k,n]·min(AlT,0) = new AlT
    psum[:, 256:] = sum_k dAl[k,n]·min(AuT,0) + dAu[k,n]·relu(AuT) = new AuT
Compute dtype bf16 (fp32 PSUM accumulation); rel err vs fp32 ≈ 2.5e-3.

The bias chain and the final concretization are m=1 matvecs against the same
state tiles (mvA pairs with dbl/lower_in, mvB with dbu/upper_in). They run as
128x32 column-tiled matmuls — tile_position=(0,32g), g = chunk%4 — so four
stream concurrently in separate column groups of the PE array, and ALL of them
(8 steps x 32 + final 32) accumulate into one PSUM bank on partition rows
{0,32,64,96}; a single DVE pass at the end sums the four rows and adds b.
This costs ~8 serialized matmul slots per step instead of 16 (fp8 DoubleRow)
or 32 (naive), and needs no fp8 shadow state.

PE work per core: 4096 main matmuls x ~220 ns (N=512 stream at 2.4 GHz + NX
dispatch) + ~160 col-tiled matvecs in ~40 4-way groups ≈ 910 µs warm;
runs land ~1.13 ms when the chip drops to its 2.0 GHz P0 power state.
"""

import numpy as np
import ml_dtypes

L = 8
D = 2048
NCORES = 8
RPC = D // NCORES  # 256 rows per core
P = 128
KC = D // P  # 16 partition chunks
W = 2 * RPC  # 512: concatenated moving width

BF16 = ml_dtypes.bfloat16

_nc_cache = {}


def _build():
    from concourse import bacc
    import concourse.tile as tile
    import concourse.mybir as mybir

    dt = mybir.dt
    nc = bacc.Bacc()

    at0 = nc.dram_tensor("at0", [KC, P, RPC], dt.bfloat16, kind="ExternalInput")
    hist = nc.dram_tensor("hist", [L, KC // 2, 2, 2, P, D], dt.bfloat16, kind="ExternalInput")
    # hbv[p, (s*2+f)*KC + i] = (dbl if f==0 else dbu)[s, i*128+p]: per-chunk
    # bias-vector columns used as m=1 stationary weights.
    hbv = nc.dram_tensor("hbv", [P, L * 2 * KC], dt.bfloat16, kind="ExternalInput")
    fin = nc.dram_tensor("fin", [P, 2 * KC], dt.bfloat16, kind="ExternalInput")
    b2 = nc.dram_tensor("b2", [1, W], dt.float32, kind="ExternalInput")
    out = nc.dram_tensor("out", [1, W], dt.float32, kind="ExternalOutput")

    with tile.TileContext(nc) as tc:
        with (
            tc.tile_pool(name="state", bufs=1) as spool,
            tc.tile_pool(name="wts", bufs=4) as wpool,
            tc.tile_pool(name="consts", bufs=1) as cpool,
            tc.tile_pool(name="bias", bufs=1) as bpool,
            tc.tile_pool(name="psum", bufs=7, space="PSUM") as ppool,
            tc.tile_pool(name="psumb", bufs=1, space="PSUM") as pbpool,
        ):
            mvA = [spool.tile([P, KC * W], dt.bfloat16, tag=f"mvA{i}", name=f"mvA{i}") for i in range(2)]
            mvB = [spool.tile([P, KC * W], dt.bfloat16, tag=f"mvB{i}", name=f"mvB{i}") for i in range(2)]
            hbvt = cpool.tile([P, L * 2 * KC], dt.bfloat16, tag="hbvt")
            fint = cpool.tile([P, 2 * KC], dt.bfloat16, tag="fint")
            b2t = bpool.tile([1, W], dt.float32, tag="b2t")

            # One PSUM bank accumulates every m=1 matvec of the kernel (bias
            # chain + final concretization) on partition rows {0,32,64,96}.
            pbias = pbpool.tile([P, W], dt.float32, tag="pb", name="pb")

            # PE warmup: a few cheap matmuls on a zeroed tile bridge the
            # initial DMA window without delaying the first real matmul.
            warm = cpool.tile([P, W], dt.bfloat16, tag="warm")
            nc.vector.memset(warm[:], 0.0)
            pw = ppool.tile([P, W], dt.float32, tag="ps", name="pw")
            for i in range(28):
                nc.tensor.matmul(pw[:, :P], warm[:, :P], warm[:, :P], start=True, stop=True)

            # Startup loads: state chunk-pairs get the sync queue to
            # themselves (each dma_start costs ~0.7 µs of sequencer dispatch,
            # and a pair lands about every 0.7 µs — matching the PE's step-0
            # consumption rate); the first two stripes and the consts go on
            # gpsimd. GpSimd compute is useless here (~4.6 µs per clamp), so
            # the step-0 clamps split relu→ScalarE / min→DVE instead.
            stg = cpool.tile([P, KC, RPC], dt.bfloat16, tag="stg", name="stg")
            stripes = {}
            st00 = wpool.tile([P, 2, 2, D], dt.bfloat16, tag="stripe", name="stripe")
            stripes[(0, 0)] = st00
            h00 = hist[0, 0]
            st01 = wpool.tile([P, 2, 2, D], dt.bfloat16, tag="stripe", name="stripe")
            stripes[(0, 1)] = st01

            def load_pair(q, eng):
                eng.dma_start(
                    stg[:, 2 * q : 2 * (q + 1), :],
                    at0[2 * q : 2 * (q + 1)].rearrange("k p r -> p k r"),
                )

            # Everything group 0 needs (state pairs + st00's jh=0 half, 2MB)
            # dispatches first, in first-use order; jh=1, st01 and the consts
            # follow, so their descriptors can't starve the critical loads.
            def load_st00(jh, half):
                sl = slice(half * D // 2, (half + 1) * D // 2)
                nc.sync.dma_start(
                    st00[:, jh, :, sl],
                    h00[jh, :, :, sl].rearrange("t p f -> p t f"),
                )

            load_st00(0, 0)
            load_pair(0, nc.sync)
            load_pair(1, nc.gpsimd)
            load_pair(2, nc.sync)
            load_pair(3, nc.gpsimd)
            load_st00(0, 1)
            load_pair(4, nc.sync)
            load_pair(5, nc.gpsimd)
            load_pair(6, nc.sync)
            load_pair(7, nc.gpsimd)
            load_st00(1, 0)
            load_st00(1, 1)
            nc.gpsimd.dma_start(st01[:], hist[0, 1].rearrange("jh t p f -> p jh t f"))
            nc.gpsimd.dma_start(hbvt[:], hbv[:])
            nc.gpsimd.dma_start(fint[:], fin[:])
            nc.gpsimd.dma_start(b2t[:], b2[:])

            # Step-0 state: Al = Au = A, so only mvA = [relu(AT) | min(AT,0)]
            # is materialized. ScalarE (relu, ~540ns/op) and DVE (min,
            # ~220ns/op) split the chain so both finish together.
            relu_f = mybir.ActivationFunctionType.Relu
            for i in range(KC):
                o = i * W
                s_i = stg[:, i, :]
                if i < 9:
                    nc.scalar.activation(mvA[0][:, o : o + RPC], s_i, relu_f)
                else:
                    nc.vector.tensor_scalar_max(mvA[0][:, o : o + RPC], s_i, 0.0)
                nc.vector.tensor_scalar_min(mvA[0][:, o + RPC : o + W], s_i, 0.0)

            for s in range(L):
                cur, nxt = s % 2, (s + 1) % 2
                A, B = mvA[cur], mvB[cur]
                An, Bn = mvA[nxt], mvB[nxt]
                for jp in range(KC // 2):
                    if (s, jp) in stripes:
                        stripe = stripes.pop((s, jp))
                    else:
                        stripe = wpool.tile([P, 2, 2, D], dt.bfloat16, tag="stripe", name="stripe")
                        nc.sync.dma_start(
                            stripe[:], hist[s, jp].rearrange("jh t p f -> p jh t f")
                        )
                    for jh in range(2):
                        j = 2 * jp + jh
                        ps = ppool.tile([P, W], dt.float32, tag="ps", name="ps")
                        for i in range(KC):
                            wA = stripe[:, jh, 0, i * P : (i + 1) * P]
                            wB = stripe[:, jh, 1, i * P : (i + 1) * P]
                            if s == 0:
                                # mvB isn't materialized at step 0 (Al = Au):
                                # the B-family reads mvA's halves swapped via
                                # two n=256 matmuls. The i==KC-1 A-matmul is
                                # reordered last to carry the full-width stop.
                                relu_h = A[:, i * W : i * W + RPC]
                                min_h = A[:, i * W + RPC : (i + 1) * W]
                                mms = [
                                    (ps[:], wA, A[:, i * W : (i + 1) * W], i == 0, i == KC - 1),
                                    (ps[:, :RPC], wB, min_h, False, False),
                                    (ps[:, RPC:], wB, relu_h, False, False),
                                ]
                                if i == KC - 1:
                                    mms = mms[1:] + mms[:1]
                                for o_ap, w_ap, r_ap, st, sp in mms:
                                    nc.tensor.matmul(o_ap, w_ap, r_ap, start=st, stop=sp)
                            else:
                                nc.tensor.matmul(
                                    ps[:],
                                    wA,
                                    A[:, i * W : (i + 1) * W],
                                    start=(i == 0),
                                    stop=False,
                                )
                                nc.tensor.matmul(
                                    ps[:],
                                    wB,
                                    B[:, i * W : (i + 1) * W],
                                    start=False,
                                    stop=(i == KC - 1),
                                )
                        h = RPC
                        o = j * W
                        nc.vector.tensor_scalar_max(An[:, o : o + h], ps[:, :h], 0.0)
                        nc.vector.tensor_scalar_min(Bn[:, o : o + h], ps[:, :h], 0.0)
                        nc.vector.tensor_scalar_max(Bn[:, o + h : o + W], ps[:, h:], 0.0)
                        nc.vector.tensor_scalar_min(An[:, o + h : o + W], ps[:, h:], 0.0)
                # bias chain: column-tiled m=1 matvecs, four concurrent in
                # separate 32-column PE groups, accumulating into pbias rows
                # {0,32,64,96}. A-family (rhs mvA) pairs with dbl, B-family
                # (rhs mvB) with dbu.
                for f, rhs_t in enumerate((A, B)):
                    base = (s * 2 + f) * KC
                    for i in range(KC):
                        g = 32 * (i % 4)
                        vcol = hbvt[:, base + i : base + i + 1]
                        if s == 0 and f == 1:
                            nc.tensor.matmul(
                                pbias[g : g + 1, :RPC],
                                vcol,
                                A[:, i * W + RPC : (i + 1) * W],
                                start=False, stop=False, tile_position=(0, g),
                            )
                            nc.tensor.matmul(
                                pbias[g : g + 1, RPC:],
                                vcol,
                                A[:, i * W : i * W + RPC],
                                start=False, stop=False, tile_position=(0, g),
                            )
                        else:
                            nc.tensor.matmul(
                                pbias[g : g + 1, :],
                                vcol,
                                rhs_t[:, i * W : (i + 1) * W],
                                start=(s == 0 and f == 0 and i < 4),
                                stop=False,
                                tile_position=(0, g),
                            )

            # final concretization against the input box, same col-tiled
            # accumulation: mvA pairs with lower_in, mvB with upper_in.
            Af, Bf = mvA[L % 2], mvB[L % 2]
            for f, rhs_t in enumerate((Af, Bf)):
                for i in range(KC):
                    g = 32 * (i % 4)
                    nc.tensor.matmul(
                        pbias[g : g + 1, :],
                        fint[:, f * KC + i : f * KC + i + 1],
                        rhs_t[:, i * W : (i + 1) * W],
                        start=False,
                        stop=(f == 1 and i >= KC - 4),
                        tile_position=(0, g),
                    )

            # res = sum of the four accumulator rows + b (one PSUM operand
            # per DVE instruction)
            acc = bpool.tile([1, W], dt.float32, tag="acc")
            res = bpool.tile([1, W], dt.float32, tag="res")
            nc.vector.tensor_add(acc[:], b2t[:], pbias[0:1, :])
            nc.vector.tensor_add(acc[:], acc[:], pbias[32:33, :])
            nc.vector.tensor_add(acc[:], acc[:], pbias[64:65, :])
            nc.vector.tensor_add(res[:], acc[:], pbias[96:97, :])
            nc.sync.dma_start(out[:], res[:])

    nc.finalize()
    return nc


def _get_nc():
    if "nc" not in _nc_cache:
        _nc_cache["nc"] = _build()
    return _nc_cache["nc"]


def _prep_inputs(A, b, hist_Al, hist_Au, hist_bl, hist_bu, lower_in, upper_in):
    A = np.asarray(A, dtype=np.float32)
    b = np.asarray(b, dtype=np.float32)
    hal = np.asarray(hist_Al, dtype=np.float32)[::-1]
    hau = np.asarray(hist_Au, dtype=np.float32)[::-1]
    hbl = np.asarray(hist_bl, dtype=np.float32)[::-1]
    hbu = np.asarray(hist_bu, dtype=np.float32)[::-1]
    lower_in = np.asarray(lower_in, dtype=np.float32)
    upper_in = np.asarray(upper_in, dtype=np.float32)

    # hist[s, j, t, p, i*P + n] = h_t[s, i*P + p, j*P + n], paired over j
    hist = np.empty([L, KC, 2, P, D], dtype=BF16)
    for t, h in enumerate((hal, hau)):
        hist[:, :, t] = (
            h.reshape(L, KC, P, KC, P).transpose(0, 3, 2, 1, 4).reshape(L, KC, P, D)
        )
    hist = hist.reshape(L, KC // 2, 2, 2, P, D)

    # hbv[p, (s*2+f)*KC + i] = (dbl, dbu)[f][s, i*P + p]
    hbv = (
        np.stack([hbl, hbu], axis=1)  # [L, 2, D]
        .reshape(L * 2 * KC, P)
        .T.astype(BF16)
    )
    hbv = np.ascontiguousarray(hbv)

    # fin[p, t*KC + i]: t=0 lower_in, t=1 upper_in
    fin = (
        np.stack([lower_in.reshape(KC, P), upper_in.reshape(KC, P)], axis=0)
        .transpose(2, 0, 1)
        .reshape(P, 2 * KC)
        .astype(BF16)
    )

    in_maps = []
    for c in range(NCORES):
        At = np.ascontiguousarray(A[c * RPC : (c + 1) * RPC].T)  # [D, RPC]
        at0 = At.reshape(KC, P, RPC).astype(BF16)
        b_blk = b[c * RPC : (c + 1) * RPC]
        b2 = np.concatenate([b_blk, b_blk]).reshape(1, W).astype(np.float32)
        in_maps.append(
            {
                "at0": at0,
                "hist": hist,
                "hbv": hbv,
                "fin": fin,
                "b2": b2,
            }
        )
    return in_maps


def _run(in_maps, trace=False):
    from concourse.bass_utils import run_bass_kernel_spmd

    nc = _get_nc()
    return run_bass_kernel_spmd(
        nc, in_maps, core_ids=list(range(NCORES)), trace=trace
    )


def kernel(A, b, hist_Al, hist_Au, hist_bl, hist_bu, lower_in, upper_in):
    in_maps = _prep_inputs(
        A, b, hist_Al, hist_Au, hist_bl, hist_bu, lower_in, upper_in
    )
    res = _run(in_maps, trace=False)
    lower = np.concatenate([res.results[c]["out"][0, :RPC] for c in range(NCORES)])
    upper = np.concatenate([res.results[c]["out"][0, RPC:] for c in range(NCORES)])
    return lower.astype(np.float32), upper.astype(np.float32)
